# revision 13
# baseline (speedup 1.0000x reference)
"""Trainium2 Bass kernel for ChannelDepsModule (sequential channel recurrence).

Math (per pixel, fp32):
    m_0 = mix_0 ; ybar_0 = round(x_0 - m_0) + m_0
    for i in 1..191:
        m_i = sum_{c<i} Wm[i-1,c] * ybar_c + b[i-1] + mix_i
        ybar_i = round(x_i - m_i) + m_i
    outputs: ybar, mix_out (= m)

Device strategy (per core, one batch image, 4096 pixels):
  - pixels on SBUF partitions ([128] x 32 chunks), channels on the free
    dim; SBUF free layout is (block, chunk, channel) so one block's
    4-chunk column group is contiguous (single-free-dim transpose APs,
    4KB-run DMAs via host-side slab permutation)
  - pixel chunks split into 2 waves of 16; the wave pipeline overlaps
    wave A's sequential DVE scan with wave B's TensorE/ScalarE tail
    (transpose -> de-interleave -> next block's P matmuls -> q prefill)
  - channels in 6 blocks of 32:
      * cross-block mix contributions P via one TensorE matmul per
        chunk (stationary ybar in channel-partition layout x Wm^T
        slice, contract = all decoded channels), pixel-partition PSUM out;
        q = x - mix - b - P is written into the block's ybar columns
      * in-block recurrence: one fused DVE scan per channel computes
        t_i = q_i - sum_j w_ij y_j directly (weights negated, +1 planted on
        the diagonal so the prefilled q column enters the dot), then one
        fused DVE op assembles y_i = round(t_i) + (x_i - t_i) using the
        +-1.5*2^23 magic constant (IEEE RNE == jnp.round)
      * mix_out column m_i = x_i - t_i is produced on the GpSimd engine,
        off the critical path
      * finished ybar columns are TensorE-transposed 4 chunks per
        instruction into chunk-interleaved PSUM, then de-interleaved into
        channel-partition tiles by [32,128] ScalarE/GpSimd copies
  - b is folded into mix on the host; mix_out channel 0 restored on host
"""

import sys

import numpy as np

if "/opt/trn_rl_repo" not in sys.path:
    sys.path.insert(0, "/opt/trn_rl_repo")

N, C, H, Wd = 8, 192, 64, 64
NPIX = H * Wd          # 4096 pixels per core
B = 32                 # channel block size
NBLK = C // B          # 6
ROUND_C = 1.5 * 2.0**23  # fp32 add of this rounds to nearest-even integer

_CACHE = {}
_DVE_OPS = {}


def _register_dve_ops():
    """Define + register the two fused DVE ops (idempotent)."""
    if _DVE_OPS:
        return _DVE_OPS
    import concourse.dve_ops as dops
    import concourse.dve_spec as ds
    from concourse.dve_spec import AluOp, Spec, Src0, Src1
    from concourse.dve_ops import CUSTOM_DVE_SPECS, OPS, DveOp
    from concourse.dve_uop import DveOpSpec

    # The stock segmented-scan machinery only implements the page-counter
    # mode; add the documented per-page *reset* behavior for scans marked
    # with `_page_reset`: at each SUB_DIM_DONE the STEP state computes
    # d <- op(init, expr) instead of op(CURR, expr).
    if not getattr(ds, "_page_reset_patched", False):
        _orig = ds._scan_overrides

        def _patched(scans, node_stage):
            seed, step = _orig(scans, node_stage)
            for sc in scans:
                if getattr(sc, "_page_reset", False):
                    d = node_stage[sc]
                    step[d] = ds._Stage(sc.op, ds._scan_init(sc), sc.expr)
            return seed, step

        ds._scan_overrides = _patched
        ds._page_reset_patched = True

    def _chaindot_ref(in0, in1, s0, s1, imm2):
        p = in0.shape[0]
        inner = in0.shape[-1]
        a = in0.reshape(p, -1, inner).astype(np.float32)
        bb = in1.reshape(p, -1, inner).astype(np.float32)
        return np.cumsum(a * bb, axis=-1, dtype=np.float32).reshape(in0.shape)

    sc = ds.scan(AluOp.ADD, Src0 * Src1)
    object.__setattr__(sc, "_page_reset", True)
    spec_cd = Spec(body=sc, reference=_chaindot_ref)

    def _quanty_ref(in0, in1, s0, s1, imm2):
        c = np.float32(s0)
        t = in0.astype(np.float32)
        return ((t + c) - c) + (in1.astype(np.float32) - t)

    spec_qy = Spec(
        body=((Src0 + ds.C0) - ds.C0) + (Src1 - Src0), reference=_quanty_ref
    )

    def _mk(name, spec, subdim):
        if any(o.name == name for o in OPS):
            op = next(o for o in OPS if o.name == name)
        else:
            shas = {}
            for ver in ("v3", "v4"):
                shas[ver] = DveOpSpec(
                    name=name, uops=ds.lower(spec, ver=ver)
                ).sha(ver)
            op = DveOp(name, spec, subdim=subdim, uops_sha=shas)
            OPS.append(op)
            CUSTOM_DVE_SPECS[name] = spec
            dops._SUB_OPCODE_FOR_NAME[name] = dops._CUSTOM_DVE_ROW_BASE + len(OPS) - 1
        return op

    _DVE_OPS["chaindot"] = _mk("CHAINDOT_SEQ_ANT", spec_cd, subdim=True)
    _DVE_OPS["quanty"] = _mk("QUANTY_ANT", spec_qy, subdim=False)
    return _DVE_OPS


def _build(n_chunks):
    """Build + compile the per-core Bass module. n_chunks pixel chunks of 128."""
    import concourse.bacc as bacc
    import concourse.mybir as mybir
    from concourse.tile import TileContext

    ops = _register_dve_ops()
    npix = n_chunks * 128
    fp32 = mybir.dt.float32
    K = n_chunks  # pixel chunks

    nc = bacc.Bacc(None, target_bir_lowering=False)

    # dram layout: row (b, p) holds the [K, B] slab of block b, lane p —
    # 4KB contiguous runs on both sides of the DMA, and block-columns of
    # any 4 chunks are contiguous in SBUF (single-free-dim transpose APs)
    xt = nc.dram_tensor("xt", [NBLK * 128, K * B], fp32, kind="ExternalInput")
    mixt = nc.dram_tensor("mixt", [NBLK * 128, K * B], fp32, kind="ExternalInput")
    wt = nc.dram_tensor("wt", [C, C], fp32, kind="ExternalInput")
    wtri = nc.dram_tensor(
        "wtri", [128, NBLK * B * B], fp32, kind="ExternalInput"
    )
    ident = nc.dram_tensor("ident", [128, 128], fp32, kind="ExternalInput")
    yt = nc.dram_tensor("yt", [NBLK * 128, K * B], fp32, kind="ExternalOutput")
    mot = nc.dram_tensor("mot", [NBLK * 128, K * B], fp32, kind="ExternalOutput")

    with TileContext(nc) as tc:
        with (
            tc.tile_pool(name="big", bufs=1) as big,
            tc.tile_pool(name="small", bufs=1) as small,
            tc.tile_pool(name="tp", bufs=2) as tp,
            tc.tile_pool(name="psum_e", bufs=4, space="PSUM") as psum_e,
            tc.tile_pool(name="psumt", bufs=2, space="PSUM") as psumt,
        ):
            # pixel-partition tiles, free layout = k*192 + c
            X = big.tile([128, K * C], fp32, tag="X")
            MIX = big.tile([128, K * C], fp32, tag="MIX")  # becomes mix_out
            XMB = big.tile([128, K * C], fp32, tag="XMB")
            Y = big.tile([128, K * C], fp32, tag="Y")
            # channel-partition decoded ybar: chans 0-127 / 128-159
            ysb_lo = big.tile([128, npix], fp32, tag="ysb_lo")
            ysb_hi = big.tile([32, npix], fp32, tag="ysb_hi")

            wt_lo = small.tile([128, C], fp32, tag="wt_lo")
            wt_hi = small.tile([64, C], fp32, tag="wt_hi")
            wtri_b = small.tile([128, NBLK * B * B], fp32, tag="wtri_b")
            id_t = small.tile([128, 128], fp32, tag="ident")

            def big_in(tile, dram):
                nc.sync.dma_start(
                    tile[:].rearrange("p (b f) -> p b f", f=K * B),
                    dram[:].rearrange("(b p) f -> p b f", p=128),
                )

            def blk_in(tile, dram, sb):
                nc.sync.dma_start(
                    tile[:, sb * K * B : (sb + 1) * K * B],
                    dram[sb * 128 : (sb + 1) * 128, :],
                )

            # small params + block 0 slabs first so the first scan starts asap
            nc.sync.dma_start(id_t[:], ident[:])
            nc.sync.dma_start(wt_lo[:], wt[0:128, :])
            nc.sync.dma_start(wt_hi[:], wt[128:C, :])
            nc.sync.dma_start(
                wtri_b[:, 0 : B * B], wtri[:, 0 : B * B]
            )
            blk_in(X, xt, 0)
            blk_in(MIX, mixt, 0)
            for sb in range(1, NBLK):
                nc.sync.dma_start(
                    wtri_b[:, sb * B * B : (sb + 1) * B * B],
                    wtri[:, sb * B * B : (sb + 1) * B * B],
                )
                blk_in(X, xt, sb)
                blk_in(MIX, mixt, sb)

            W = 2           # pixel waves: wave A's scan hides wave B's tail
            KW = K // W
            FB = K * B      # free-size of one block slab

            def col(tile, ch, w):  # strided [128, KW] view of channel ch
                b, c = divmod(ch, B)
                return tile[:].rearrange("p (b k c) -> p b k c", c=B, k=K)[
                    :, b, w * KW : (w + 1) * KW, c
                ]

            def ycols(sb, w, j0, j1):  # [128, KW, j1-j0] view of block cols
                return Y[:].rearrange("p (b k c) -> p b k c", c=B, k=K)[
                    :, sb, w * KW : (w + 1) * KW, j0:j1
                ]

            def xmb_slice(sb, w):
                return XMB[:].rearrange("p (b k c) -> p b k c", c=B, k=K)[
                    :, sb, w * KW : (w + 1) * KW, :
                ]

            def xmb_prep(sb):  # XMB = X - (MIX + b) for one block slab
                s = slice(sb * FB, (sb + 1) * FB)
                nc.gpsimd.tensor_sub(XMB[:, s], X[:, s], MIX[:, s])

            def p_phase1(t, w, pp):
                """Early contributions to target block t: contract all rows of
                blocks 0..t-2 (rows 0..32(t-1)). Issued right after block
                t-2's transpose, a full block-scan before q_fin(t) needs it."""
                kdec = (t - 1) * B
                for kk in range(KW):
                    k = w * KW + kk
                    nc.tensor.matmul(
                        pp[:, kk * B : (kk + 1) * B],
                        ysb_lo[0:kdec, k * 128 : (k + 1) * 128],
                        wt_lo[0:kdec, t * B : (t + 1) * B],
                        start=(kk == 0),
                        stop=False,
                    )

            def p_phase2(t, w, pp, fresh):
                """Late contribution to target t: block t-1's 32 fresh rows."""
                r0 = (t - 1) * B
                if r0 < 128:
                    src, wsrc, s0 = ysb_lo, wt_lo, r0
                else:
                    src, wsrc, s0 = ysb_hi, wt_hi, r0 - 128
                tpos = (96, 0) if s0 == 96 else None
                for kk in range(KW):
                    k = w * KW + kk
                    nc.tensor.matmul(
                        pp[:, kk * B : (kk + 1) * B],
                        src[s0 : s0 + B, k * 128 : (k + 1) * 128],
                        wsrc[s0 : s0 + B, t * B : (t + 1) * B],
                        start=fresh and (kk == 0),
                        stop=(kk == KW - 1),
                        tile_position=tpos,
                    )

            def q_fin(sb, w, pp):
                """Block sb's Y cols = XMB - PP."""
                nc.vector.tensor_sub(
                    ycols(sb, w, 0, B),
                    xmb_slice(sb, w),
                    pp[:].rearrange("p (k c) -> p k c", c=B),
                )

            def transpose_block(sb, w):
                """Transpose Y cols of block sb, wave w into ysb (chan-part).

                4 pixel-chunks per TensorE transpose into one PSUM supertile;
                the chunk-interleaved result is de-interleaved by four wide
                [32, 4, 128] ScalarE copies (one per chunk-in-group residue).
                """
                base = sb * B
                if base < 128:
                    dst, dr0 = ysb_lo, base
                else:
                    dst, dr0 = ysb_hi, base - 128
                slab = Y[:].rearrange("p (b f) -> p b f", f=FB)[:, sb, :]
                ng = KW // 4
                pt = psumt.tile([128, ng * 128], fp32, tag="pt")
                for gi in range(ng):
                    g = w * KW + gi * 4
                    nc.tensor.transpose(
                        pt[:, gi * 128 : (gi + 1) * 128],
                        slab[:, g * B : (g + 4) * B],
                        id_t[:],
                    )
                dstk = dst[dr0 : dr0 + B, :].rearrange(
                    "p (g j x) -> p g j x", j=4, x=128
                )
                ptk = pt[:].rearrange("p (a x) -> p a x", x=128)
                for j in range(4):
                    nc.scalar.copy(
                        dstk[:, w * ng : (w + 1) * ng, j, :],
                        ptk[j * B : (j + 1) * B, :, :],
                    )

            def steps(sb, w, pp):
                base = sb * B
                T = tp.tile([128, B * KW], fp32, tag="T")
                Tk = T[:].rearrange("p (k c) -> p k c", c=B)
                for i in range(B):
                    ch = base + i
                    if i > 0:
                        # scan's stride-0 inner out: final cumsum value (t_i)
                        # lands in T column i (last-wins)
                        woff = sb * B * B + i * B
                        wrow = (
                            wtri_b[:, woff : woff + i + 1]
                            .unsqueeze(1)
                            .broadcast_to([128, KW, i + 1])
                        )
                        nc.vector._custom_dve(
                            ops["chaindot"],
                            out=Tk[:, :, i : i + 1].broadcast_to([128, KW, i + 1]),
                            in0=ycols(sb, w, 0, i + 1), in1=wrow,
                        )
                        t_ap = Tk[:, :, i]
                    else:
                        t_ap = col(Y, ch, w)
                    # y = round(t) + (x - t)
                    nc.vector._custom_dve(
                        ops["quanty"], out=col(Y, ch, w), in0=t_ap,
                        in1=col(X, ch, w), s0=ROUND_C,
                    )
                # mix_out for the whole block in bulk, off the critical path:
                #   cols 1..31: m = x - t (t persisted in T)
                #   col 0:      m = (mix + b) + P  (no in-block terms)
                xk = X[:].rearrange("p (b k c) -> p b k c", c=B, k=K)[
                    :, sb, w * KW : (w + 1) * KW, 1:B
                ]
                mk = MIX[:].rearrange("p (b k c) -> p b k c", c=B, k=K)[
                    :, sb, w * KW : (w + 1) * KW, 1:B
                ]
                nc.gpsimd.tensor_sub(mk, xk, Tk[:, :, 1:B])
                if sb > 0:
                    # GpSimd can't read PSUM; this one stays on DVE
                    m0 = col(MIX, base, w)
                    nc.vector.tensor_add(
                        m0, m0, pp[:].rearrange("p (k c) -> p k c", c=B)[:, :, 0]
                    )

            def blk_out(dram, tile, sb):
                nc.sync.dma_start(
                    dram[sb * 128 : (sb + 1) * 128, :],
                    tile[:, sb * FB : (sb + 1) * FB],
                )

            # ---------------- schedule ----------------
            # Wave pipeline: wave w's scan overlaps the other wave's
            # transpose -> p_all -> q_fin tail on Tensor/Scalar.
            # pp_ready[w]: fully-accumulated P for the block about to scan.
            # pp_stage[w]: phase1-accumulated P for the block after that.
            pp_ready = [None] * W
            pp_stage = [None] * W
            for sb in range(NBLK):
                if sb + 1 < NBLK:
                    xmb_prep(sb + 1)  # one-block lookahead on GpSimd
                for w in range(W):
                    pp_used = pp_ready[w]
                    if sb == 0:
                        # q for block 0 directly on DVE (no XMB round-trip)
                        nc.vector.tensor_sub(
                            ycols(sb, w, 0, B),
                            X[:].rearrange("p (b k c) -> p b k c", c=B, k=K)[
                                :, 0, w * KW : (w + 1) * KW, :
                            ],
                            MIX[:].rearrange("p (b k c) -> p b k c", c=B, k=K)[
                                :, 0, w * KW : (w + 1) * KW, :
                            ],
                        )
                    else:
                        q_fin(sb, w, pp_used)
                    steps(sb, w, pp_used)
                    if sb + 1 < NBLK:
                        transpose_block(sb, w)
                        # finish next block's P (urgent)...
                        if sb == 0:
                            pp = psum_e.tile([128, B * KW], fp32, tag="pp")
                            p_phase2(1, w, pp, fresh=True)
                        else:
                            pp = pp_stage[w]
                            p_phase2(sb + 1, w, pp, fresh=False)
                        pp_ready[w] = pp
                        # ...then start the block-after-next's P (slack-filler)
                        if sb + 2 < NBLK:
                            pps = psum_e.tile([128, B * KW], fp32, tag="pp")
                            p_phase1(sb + 2, w, pps)
                            pp_stage[w] = pps
                blk_out(yt, Y, sb)
                blk_out(mot, MIX, sb)

    nc.compile()
    return nc


def get_nc(n_chunks=NPIX // 128):
    if n_chunks not in _CACHE:
        _CACHE[n_chunks] = _build(n_chunks)
    return _CACHE[n_chunks]


def make_core_inputs(x, mix, W, b):
    """Host-side layout prep. Returns list of per-core input dicts."""
    Wm = (W * np.tril(np.ones((C - 1, C), np.float32))).astype(np.float32)
    wt = np.zeros((C, C), np.float32)
    wt[:, 1:] = Wm.T  # wt[c, i] = Wm[i-1, c]
    # in-block triangle, negated, with +1 on the diagonal: the scan over
    # [y_0..y_{i-1}, q_i] then yields t_i = q_i - sum_j w_ij y_j directly
    wtri = np.zeros((NBLK, B, B), np.float32)
    for sb in range(NBLK):
        for i in range(1, B):
            ch = sb * B + i
            wtri[sb, i, :i] = -Wm[ch - 1, sb * B : sb * B + i]
            wtri[sb, i, i] = 1.0
    wtri = np.ascontiguousarray(
        np.broadcast_to(wtri.reshape(1, -1), (128, NBLK * B * B))
    )
    bpad = np.zeros((C,), np.float32)
    bpad[1:] = b
    ident = np.eye(128, dtype=np.float32)

    def to_slab(a):  # [C, H, W] -> [(b p), (k c)]
        return np.ascontiguousarray(
            a.reshape(NBLK, B, NPIX // 128, 128).transpose(0, 3, 2, 1)
        ).reshape(NBLK * 128, -1)

    in_maps = []
    for n in range(N):
        xtn = to_slab(x[n])
        mixn = to_slab(mix[n] + bpad[:, None, None])
        in_maps.append(
            {"xt": xtn, "mixt": mixn, "wt": wt, "wtri": wtri, "ident": ident}
        )
    return in_maps


def from_slab(a):  # [(b p), (k c)] -> [C, H, W]
    return (
        a.reshape(NBLK, 128, NPIX // 128, B)
        .transpose(0, 3, 2, 1)
        .reshape(C, H, Wd)
    )


def kernel(x, mix, W, b):
    from concourse.bass_utils import run_bass_kernel_spmd

    x = np.asarray(x, np.float32)
    mix = np.asarray(mix, np.float32)
    W = np.asarray(W, np.float32)
    b = np.asarray(b, np.float32)

    nc = get_nc()
    in_maps = make_core_inputs(x, mix, W, b)
    res = run_bass_kernel_spmd(nc, in_maps, list(range(N)))

    ybar = np.empty((N, C, H, Wd), np.float32)
    mix_out = np.empty((N, C, H, Wd), np.float32)
    for n in range(N):
        ybar[n] = from_slab(res.results[n]["yt"])
        mix_out[n] = from_slab(res.results[n]["mot"])
    mix_out[:, 0] = mix[:, 0]  # reference passes mix ch0 through exactly
    return ybar, mix_out



# revision 25
# speedup vs baseline: 1.1984x; 1.1984x over previous
"""Trainium2 Bass kernel for ChannelDepsModule (sequential channel recurrence).

Math (per pixel, fp32):
    m_0 = mix_0 ; ybar_0 = round(x_0 - m_0) + m_0
    for i in 1..191:
        m_i = sum_{c<i} Wm[i-1,c] * ybar_c + b[i-1] + mix_i
        ybar_i = round(x_i - m_i) + m_i
    outputs: ybar, mix_out (= m)

Device strategy (per core, one batch image, 4096 pixels):
  - pixels on SBUF partitions ([128] x 32 chunks), channels on the free
    dim; SBUF free layout is (block, chunk, channel) so one block's
    4-chunk column group is contiguous (single-free-dim transpose APs,
    4KB-run DMAs via host-side slab permutation)
  - pixel chunks split into 2 waves of 16; the wave pipeline overlaps
    wave A's sequential DVE scan with wave B's TensorE/ScalarE tail
    (transpose -> de-interleave -> next block's P matmuls -> q prefill)
  - channels in 6 blocks of 32:
      * cross-block mix contributions P via one TensorE matmul per
        chunk (stationary ybar in channel-partition layout x Wm^T
        slice, contract = all decoded channels), pixel-partition PSUM out;
        q = x - mix - b - P is written into the block's ybar columns
      * in-block recurrence: one fused DVE scan per channel computes
        t_i = q_i - sum_j w_ij y_j directly (weights negated, +1 planted on
        the diagonal so the prefilled q column enters the dot), then one
        fused DVE op assembles y_i = round(t_i) + (x_i - t_i) using the
        +-1.5*2^23 magic constant (IEEE RNE == jnp.round)
      * mix_out column m_i = x_i - t_i is produced on the GpSimd engine,
        off the critical path
      * finished ybar columns are TensorE-transposed 4 chunks per
        instruction into chunk-interleaved PSUM, then de-interleaved into
        channel-partition tiles by [32,128] ScalarE/GpSimd copies
  - b is folded into mix on the host; mix_out channel 0 restored on host
"""

import sys

import numpy as np

if "/opt/trn_rl_repo" not in sys.path:
    sys.path.insert(0, "/opt/trn_rl_repo")

N, C, H, Wd = 8, 192, 64, 64
NPIX = H * Wd          # 4096 pixels per core
B = 32                 # channel block size
NBLK = C // B          # 6
ROUND_C = 1.5 * 2.0**23  # fp32 add of this rounds to nearest-even integer

_CACHE = {}
_DVE_OPS = {}


def _register_dve_ops():
    """Define + register the two fused DVE ops (idempotent)."""
    if _DVE_OPS:
        return _DVE_OPS
    import concourse.dve_ops as dops
    import concourse.dve_spec as ds
    from concourse.dve_spec import AluOp, Spec, Src0, Src1
    from concourse.dve_ops import CUSTOM_DVE_SPECS, OPS, DveOp
    from concourse.dve_uop import DveOpSpec

    # The stock segmented-scan machinery only implements the page-counter
    # mode; add the documented per-page *reset* behavior for scans marked
    # with `_page_reset`: at each SUB_DIM_DONE the STEP state computes
    # d <- op(init, expr) instead of op(CURR, expr).
    if not getattr(ds, "_page_reset_patched", False):
        _orig = ds._scan_overrides

        def _patched(scans, node_stage):
            seed, step = _orig(scans, node_stage)
            for sc in scans:
                if getattr(sc, "_page_reset", False):
                    d = node_stage[sc]
                    step[d] = ds._Stage(sc.op, ds._scan_init(sc), sc.expr)
            return seed, step

        ds._scan_overrides = _patched
        ds._page_reset_patched = True

    def _chaindot_ref(in0, in1, s0, s1, imm2):
        p = in0.shape[0]
        inner = in0.shape[-1]
        a = in0.reshape(p, -1, inner).astype(np.float32)
        bb = in1.reshape(p, -1, inner).astype(np.float32)
        return np.cumsum(a * bb, axis=-1, dtype=np.float32).reshape(in0.shape)

    sc = ds.scan(AluOp.ADD, Src0 * Src1)
    object.__setattr__(sc, "_page_reset", True)
    spec_cd = Spec(body=sc, reference=_chaindot_ref)

    def _quanty_ref(in0, in1, s0, s1, imm2):
        c = np.float32(s0)
        t = in0.astype(np.float32)
        return ((t + c) - c) + (in1.astype(np.float32) - t)

    spec_qy = Spec(
        body=((Src0 + ds.C0) - ds.C0) + (Src1 - Src0), reference=_quanty_ref
    )

    def _mk(name, spec, subdim):
        if any(o.name == name for o in OPS):
            op = next(o for o in OPS if o.name == name)
        else:
            shas = {}
            for ver in ("v3", "v4"):
                shas[ver] = DveOpSpec(
                    name=name, uops=ds.lower(spec, ver=ver)
                ).sha(ver)
            op = DveOp(name, spec, subdim=subdim, uops_sha=shas)
            OPS.append(op)
            CUSTOM_DVE_SPECS[name] = spec
            dops._SUB_OPCODE_FOR_NAME[name] = dops._CUSTOM_DVE_ROW_BASE + len(OPS) - 1
        return op

    _DVE_OPS["chaindot"] = _mk("CHAINDOT_SEQ_ANT", spec_cd, subdim=True)
    _DVE_OPS["quanty"] = _mk("QUANTY_ANT", spec_qy, subdim=False)
    return _DVE_OPS


def _build(n_chunks):
    """Build + compile the per-core Bass module. n_chunks pixel chunks of 128."""
    import concourse.bacc as bacc
    import concourse.mybir as mybir
    from concourse.tile import TileContext

    ops = _register_dve_ops()
    npix = n_chunks * 128
    fp32 = mybir.dt.float32
    fp16 = mybir.dt.float16
    K = n_chunks  # pixel chunks

    nc = bacc.Bacc(None, target_bir_lowering=False)

    # dram layout: row (b, p) holds the [K, B] slab of block b, lane p —
    # 4KB contiguous runs on both sides of the DMA, and block-columns of
    # any 4 chunks are contiguous in SBUF (single-free-dim transpose APs)
    xt = nc.dram_tensor("xt", [NBLK * 128, K * B], fp32, kind="ExternalInput")
    mixt = nc.dram_tensor("mixt", [NBLK * 128, K * B], fp32, kind="ExternalInput")
    # fp16 split weights: wt = wth + wtl (to fp16 ulp^2)
    wth = nc.dram_tensor("wth", [C, C], fp16, kind="ExternalInput")
    wtl = nc.dram_tensor("wtl", [C, C], fp16, kind="ExternalInput")
    wtri = nc.dram_tensor(
        "wtri", [128, NBLK * B * B], fp32, kind="ExternalInput"
    )
    ident = nc.dram_tensor("ident", [128, 128], fp16, kind="ExternalInput")
    yt = nc.dram_tensor("yt", [NBLK * 128, K * B], fp32, kind="ExternalOutput")
    mot = nc.dram_tensor("mot", [NBLK * 128, K * B], fp32, kind="ExternalOutput")

    with TileContext(nc) as tc:
        with (
            tc.tile_pool(name="big", bufs=1) as big,
            tc.tile_pool(name="small", bufs=1) as small,
            tc.tile_pool(name="tp", bufs=2) as tp,
            tc.tile_pool(name="yhp", bufs=2) as yhp,
            tc.tile_pool(name="ylp", bufs=2) as ylp,
            tc.tile_pool(name="psum_e", bufs=3, space="PSUM") as psum_e,
            tc.tile_pool(name="psumt", bufs=2, space="PSUM") as psumt,
        ):
            # pixel-partition tiles, free layout = k*192 + c
            X = big.tile([128, K * C], fp32, tag="X")
            MIX = big.tile([128, K * C], fp32, tag="MIX")  # becomes mix_out
            XMB = big.tile([128, K * C], fp32, tag="XMB")
            Y = big.tile([128, K * C], fp32, tag="Y")
            # channel-partition decoded ybar as fp16 hi/lo split pairs
            ysb_lo_h = big.tile([128, npix], fp16, tag="ysb_lo_h")
            ysb_lo_l = big.tile([128, npix], fp16, tag="ysb_lo_l")
            ysb_hi_h = big.tile([32, npix], fp16, tag="ysb_hi_h")
            ysb_hi_l = big.tile([32, npix], fp16, tag="ysb_hi_l")

            wt_lo_h = small.tile([128, C], fp16, tag="wt_lo_h")
            wt_lo_l = small.tile([128, C], fp16, tag="wt_lo_l")
            wt_hi_h = small.tile([64, C], fp16, tag="wt_hi_h")
            wt_hi_l = small.tile([64, C], fp16, tag="wt_hi_l")
            wtri_b = small.tile([128, NBLK * B * B], fp32, tag="wtri_b")
            id_t = small.tile([128, 128], fp16, tag="ident")

            def big_in(tile, dram):
                nc.sync.dma_start(
                    tile[:].rearrange("p (b f) -> p b f", f=K * B),
                    dram[:].rearrange("(b p) f -> p b f", p=128),
                )

            def blk_in(tile, dram, sb):
                nc.sync.dma_start(
                    tile[:, sb * K * B : (sb + 1) * K * B],
                    dram[sb * 128 : (sb + 1) * 128, :],
                )

            # small params + block 0 slabs first so the first scan starts asap
            nc.sync.dma_start(id_t[:], ident[:])
            nc.sync.dma_start(wt_lo_h[:], wth[0:128, :])
            nc.sync.dma_start(wt_lo_l[:], wtl[0:128, :])
            nc.sync.dma_start(wt_hi_h[:], wth[128:C, :])
            nc.sync.dma_start(wt_hi_l[:], wtl[128:C, :])
            nc.sync.dma_start(
                wtri_b[:, 0 : B * B], wtri[:, 0 : B * B]
            )
            blk_in(X, xt, 0)
            blk_in(MIX, mixt, 0)
            for sb in range(1, NBLK):
                nc.sync.dma_start(
                    wtri_b[:, sb * B * B : (sb + 1) * B * B],
                    wtri[:, sb * B * B : (sb + 1) * B * B],
                )
                blk_in(X, xt, sb)
                blk_in(MIX, mixt, sb)

            W = 2           # pixel waves: wave A's scan hides wave B's tail
            KW = K // W
            FB = K * B      # free-size of one block slab

            def col(tile, ch, w):  # strided [128, KW] view of channel ch
                b, c = divmod(ch, B)
                return tile[:].rearrange("p (b k c) -> p b k c", c=B, k=K)[
                    :, b, w * KW : (w + 1) * KW, c
                ]

            def ycols(sb, w, j0, j1):  # [128, KW, j1-j0] view of block cols
                return Y[:].rearrange("p (b k c) -> p b k c", c=B, k=K)[
                    :, sb, w * KW : (w + 1) * KW, j0:j1
                ]

            def xmb_slice(sb, w):
                return XMB[:].rearrange("p (b k c) -> p b k c", c=B, k=K)[
                    :, sb, w * KW : (w + 1) * KW, :
                ]

            def xmb_prep(sb):  # XMB = X - (MIX + b) for one block slab
                s = slice(sb * FB, (sb + 1) * FB)
                nc.gpsimd.tensor_sub(XMB[:, s], X[:, s], MIX[:, s])

            # exact-to-~2^-22 cross-block P via fp16 split matmuls:
            #   P = yh@Wh + yl@Wh + yh@Wl   (yl*Wl term negligible)
            def p_lo(sb, w, pp):
                """Contract over chans 0..min(32*sb,128) for block sb. For
                sb=5 this part skips block 4's rows, so it can be issued
                before block 4's transpose and overlap the running scan."""
                kdec = min(sb * B, 128)
                sets = [
                    (ysb_lo_h, wt_lo_h),
                    (ysb_lo_l, wt_lo_h),
                    (ysb_lo_h, wt_lo_l),
                ]
                for s, (ys, ws) in enumerate(sets):
                    for kk in range(KW):
                        k = w * KW + kk
                        nc.tensor.matmul(
                            pp[:, kk * B : (kk + 1) * B],
                            ys[0:kdec, k * 128 : (k + 1) * 128],
                            ws[0:kdec, sb * B : (sb + 1) * B],
                            start=(s == 0 and kk == 0),
                            stop=(sb * B <= 128)
                            and (s == 2 and kk == KW - 1),
                        )

            def p_hi(sb, w, pp):
                """Remaining 32-row tail (chans 128..159) for sb=5."""
                sets = [
                    (ysb_hi_h, wt_hi_h),
                    (ysb_hi_l, wt_hi_h),
                    (ysb_hi_h, wt_hi_l),
                ]
                for s, (ys, ws) in enumerate(sets):
                    for kk in range(KW):
                        k = w * KW + kk
                        nc.tensor.matmul(
                            pp[:, kk * B : (kk + 1) * B],
                            ys[0:B, k * 128 : (k + 1) * 128],
                            ws[0:B, sb * B : (sb + 1) * B],
                            start=False,
                            stop=(s == 2 and kk == KW - 1),
                        )

            def q_fin(sb, w, pp):
                """Block sb's Y cols = XMB - PP."""
                nc.vector.tensor_sub(
                    ycols(sb, w, 0, B),
                    xmb_slice(sb, w),
                    pp[:].rearrange("p (k c) -> p k c", c=B),
                )

            def transpose_block(sb, w, YH, YL):
                """Transpose the block's fp16 yh/yl slabs into ysb (chan-part).

                4 pixel-chunks per TensorE transpose into a PSUM supertile;
                the chunk-interleaved result is de-interleaved by four wide
                [32, 4, 128] ScalarE copies per tensor."""
                base = sb * B
                if base < 128:
                    dsts, dr0 = (ysb_lo_h, ysb_lo_l), base
                else:
                    dsts, dr0 = (ysb_hi_h, ysb_hi_l), base - 128
                ng = KW // 4
                for src, dst in zip((YH, YL), dsts):
                    pt = psumt.tile([128, ng * 128], fp16, tag="pt")
                    for gi in range(ng):
                        nc.tensor.transpose(
                            pt[:, gi * 128 : (gi + 1) * 128],
                            src[:, gi * 128 : (gi + 1) * 128],
                            id_t[:],
                        )
                    dstk = dst[dr0 : dr0 + B, :].rearrange(
                        "p (g j x) -> p g j x", j=4, x=128
                    )
                    ptk = pt[:].rearrange("p (a x) -> p a x", x=128)
                    for j in range(4):
                        nc.scalar.copy(
                            dstk[:, w * ng : (w + 1) * ng, j, :],
                            ptk[j * B : (j + 1) * B, :, :],
                        )

            def steps(sb, w, pp):
                base = sb * B
                T = tp.tile([128, B * KW], fp32, tag="T")
                Tk = T[:].rearrange("p (k c) -> p k c", c=B)
                for i in range(B):
                    ch = base + i
                    if i > 0:
                        # scan's stride-0 inner out: final cumsum value (t_i)
                        # lands in T column i (last-wins)
                        woff = sb * B * B + i * B
                        wrow = (
                            wtri_b[:, woff : woff + i + 1]
                            .unsqueeze(1)
                            .broadcast_to([128, KW, i + 1])
                        )
                        nc.vector._custom_dve(
                            ops["chaindot"],
                            out=Tk[:, :, i : i + 1].broadcast_to([128, KW, i + 1]),
                            in0=ycols(sb, w, 0, i + 1), in1=wrow,
                        )
                        t_ap = Tk[:, :, i]
                    else:
                        t_ap = col(Y, ch, w)
                    # y = round(t) + (x - t)
                    nc.vector._custom_dve(
                        ops["quanty"], out=col(Y, ch, w), in0=t_ap,
                        in1=col(X, ch, w), s0=ROUND_C,
                    )
                # mix_out for the whole block in bulk, off the critical path:
                #   cols 1..31: m = x - t (t persisted in T)
                #   col 0:      m = (mix + b) + P  (no in-block terms)
                xk = X[:].rearrange("p (b k c) -> p b k c", c=B, k=K)[
                    :, sb, w * KW : (w + 1) * KW, 1:B
                ]
                mk = MIX[:].rearrange("p (b k c) -> p b k c", c=B, k=K)[
                    :, sb, w * KW : (w + 1) * KW, 1:B
                ]
                nc.gpsimd.tensor_sub(mk, xk, Tk[:, :, 1:B])
                if sb > 0:
                    # GpSimd can't read PSUM; this one stays on DVE
                    m0 = col(MIX, base, w)
                    nc.vector.tensor_add(
                        m0, m0, pp[:].rearrange("p (k c) -> p k c", c=B)[:, :, 0]
                    )
                if sb + 1 == NBLK:
                    return None, None
                # fp16 hi/lo split of the freshly decoded block for TensorE
                yslab = Y[:].rearrange("p (b f) -> p b f", f=FB)[
                    :, sb, w * KW * B : (w + 1) * KW * B
                ]
                YH = yhp.tile([128, KW * B], fp16, tag="YH")
                YL = ylp.tile([128, KW * B], fp16, tag="YL")
                nc.scalar.copy(YH[:], yslab)
                nc.gpsimd.tensor_sub(YL[:], yslab, YH[:])
                return YH, YL

            def blk_out(dram, tile, sb):
                nc.sync.dma_start(
                    dram[sb * 128 : (sb + 1) * 128, :],
                    tile[:, sb * FB : (sb + 1) * FB],
                )

            # ---------------- schedule ----------------
            # Wave pipeline: wave w's scan overlaps the other wave's
            # transpose -> p_all -> q_fin tail on Tensor/Scalar.
            pp_cur = [None] * W
            for sb in range(NBLK):
                if sb + 1 < NBLK:
                    xmb_prep(sb + 1)  # one-block lookahead on GpSimd
                for w in range(W):
                    pp_used = pp_cur[w]
                    if sb == 0:
                        # q for block 0 directly on DVE (no XMB round-trip)
                        nc.vector.tensor_sub(
                            ycols(sb, w, 0, B),
                            X[:].rearrange("p (b k c) -> p b k c", c=B, k=K)[
                                :, 0, w * KW : (w + 1) * KW, :
                            ],
                            MIX[:].rearrange("p (b k c) -> p b k c", c=B, k=K)[
                                :, 0, w * KW : (w + 1) * KW, :
                            ],
                        )
                    else:
                        q_fin(sb, w, pp_used)
                    if sb + 1 == NBLK - 1:
                        # dest block 5's contract-128 part only needs blocks
                        # 0..3: issue now so it runs under this wave's scan
                        pp = psum_e.tile([128, B * KW], fp32, tag="pp")
                        p_lo(sb + 1, w, pp)
                        pp_cur[w] = pp
                    YH, YL = steps(sb, w, pp_used)
                    if sb + 1 < NBLK:
                        transpose_block(sb, w, YH, YL)
                        if sb + 1 == NBLK - 1:
                            p_hi(sb + 1, w, pp_cur[w])
                        else:
                            pp = psum_e.tile([128, B * KW], fp32, tag="pp")
                            p_lo(sb + 1, w, pp)
                            pp_cur[w] = pp
                blk_out(yt, Y, sb)
                blk_out(mot, MIX, sb)

    nc.compile()
    return nc


def get_nc(n_chunks=NPIX // 128):
    if n_chunks not in _CACHE:
        _CACHE[n_chunks] = _build(n_chunks)
    return _CACHE[n_chunks]


def make_core_inputs(x, mix, W, b):
    """Host-side layout prep. Returns list of per-core input dicts."""
    Wm = (W * np.tril(np.ones((C - 1, C), np.float32))).astype(np.float32)
    wt = np.zeros((C, C), np.float32)
    wt[:, 1:] = Wm.T  # wt[c, i] = Wm[i-1, c]
    wth = wt.astype(np.float16)
    wtl = (wt - wth.astype(np.float32)).astype(np.float16)
    # in-block triangle, negated, with +1 on the diagonal: the scan over
    # [y_0..y_{i-1}, q_i] then yields t_i = q_i - sum_j w_ij y_j directly
    wtri = np.zeros((NBLK, B, B), np.float32)
    for sb in range(NBLK):
        for i in range(1, B):
            ch = sb * B + i
            wtri[sb, i, :i] = -Wm[ch - 1, sb * B : sb * B + i]
            wtri[sb, i, i] = 1.0
    wtri = np.ascontiguousarray(
        np.broadcast_to(wtri.reshape(1, -1), (128, NBLK * B * B))
    )
    bpad = np.zeros((C,), np.float32)
    bpad[1:] = b
    ident = np.eye(128, dtype=np.float16)

    def to_slab(a):  # [C, H, W] -> [(b p), (k c)]
        return np.ascontiguousarray(
            a.reshape(NBLK, B, NPIX // 128, 128).transpose(0, 3, 2, 1)
        ).reshape(NBLK * 128, -1)

    in_maps = []
    for n in range(N):
        xtn = to_slab(x[n])
        mixn = to_slab(mix[n] + bpad[:, None, None])
        in_maps.append(
            {
                "xt": xtn,
                "mixt": mixn,
                "wth": wth,
                "wtl": wtl,
                "wtri": wtri,
                "ident": ident,
            }
        )
    return in_maps


def from_slab(a):  # [(b p), (k c)] -> [C, H, W]
    return (
        a.reshape(NBLK, 128, NPIX // 128, B)
        .transpose(0, 3, 2, 1)
        .reshape(C, H, Wd)
    )


def kernel(x, mix, W, b):
    from concourse.bass_utils import run_bass_kernel_spmd

    x = np.asarray(x, np.float32)
    mix = np.asarray(mix, np.float32)
    W = np.asarray(W, np.float32)
    b = np.asarray(b, np.float32)

    nc = get_nc()
    in_maps = make_core_inputs(x, mix, W, b)
    res = run_bass_kernel_spmd(nc, in_maps, list(range(N)))

    ybar = np.empty((N, C, H, Wd), np.float32)
    mix_out = np.empty((N, C, H, Wd), np.float32)
    for n in range(N):
        ybar[n] = from_slab(res.results[n]["yt"])
        mix_out[n] = from_slab(res.results[n]["mot"])
    mix_out[:, 0] = mix[:, 0]  # reference passes mix ch0 through exactly
    return ybar, mix_out

